# revision 50
# baseline (speedup 1.0000x reference)
"""Multi-head attention (B=2, S=2048, H=1024, 16 heads) on 8 TRN2 NeuronCores.

Sharding (tensor-parallel heads x data-parallel batch, per the hint):
  core c -> batch b = c // 4, head group g = c % 4 (4 heads each).
Each core computes, for its batch and its 4 heads:
  Q^T, K^T (transposed layout, qcol on partitions, fp16, duplicated across
  both partition halves), V^T (f32r) transposed to natural V (bf16) on the
  PE, scores^T = K^T.T @ Q^T per head with two tok_k chunks row-tiled
  concurrently in the two array halves, probs = exp(scores) (no max
  subtraction -- scores ~ N(0,1), bounded), ctx^T via an M=65 ones-augmented
  V so the softmax denominator lands in row 64 of the same accumulation,
  division by the denominator (reciprocal + gpsimd partition broadcast),
  and the partial out-projection ctx^T.T @ Wo_rows.  The 4 partial outputs
  per batch are summed on the host during unsharding (Megatron-style TP
  partial sums).

Projections run as float32r (full PE rate at N>=512); the scores operands
are fp16 and the probs/V path is bf16 (all well inside the tolerance).
Biases: bq/bk are applied on-device (per-partition adds; they are zeros in
this problem), bv/bo are folded into a host-side additive constant
(bv @ Wo + bo), which is exact.
"""

import ml_dtypes
import numpy as np

import concourse.bacc as bacc
import concourse.mybir as mybir
import concourse.tile as tile
from concourse.bass_utils import run_bass_kernel_spmd

NCORES = 8
B, S, HID = 2, 2048, 1024
NH, HD = 16, 64
HPC = 4            # heads per core
QC = HPC * HD      # 256 local projection cols per core
HC = HID // 128    # 8 hidden chunks
TC = S // 128      # 16 token chunks
TB = S // 512      # 4 token blocks

F32 = mybir.dt.float32
F32R = mybir.dt.float32r
BF16 = mybir.dt.bfloat16
FP16 = mybir.dt.float16
EXP = mybir.ActivationFunctionType.Exp
MULT = mybir.AluOpType.mult


def build_nc():
    nc = bacc.Bacc("TRN2", target_bir_lowering=False, debug=False,
                   num_devices=NCORES)
    xT = nc.declare_dram_parameter("xT", [HID, S], FP16, isOutput=False)
    wq = nc.declare_dram_parameter("wq", [HID, QC], FP16, isOutput=False)
    wk = nc.declare_dram_parameter("wk", [HID, QC], FP16, isOutput=False)
    wv = nc.declare_dram_parameter("wv", [HID, QC], FP16, isOutput=False)
    wo = nc.declare_dram_parameter("wo", [QC, HID], BF16, isOutput=False)
    bq = nc.declare_dram_parameter("bq", [QC], F32, isOutput=False)
    bk = nc.declare_dram_parameter("bk", [QC], F32, isOutput=False)
    ident = nc.declare_dram_parameter("ident", [128, 128], F32, isOutput=False)
    out = nc.declare_dram_parameter("out", [S, HID], BF16, isOutput=True)

    with tile.TileContext(nc) as tc:
        with (
            tc.tile_pool(name="const", bufs=1) as constp,
            tc.tile_pool(name="qkv", bufs=1) as qkvp,
        ):
            wo_sb = constp.tile([128, 2 * HID], BF16)
            bq_sb = constp.tile([128, 2], F32)
            bk_sb = constp.tile([128, 2], F32)
            idf_sb = constp.tile([128, 128], F32)
            idb_sb = constp.tile([128, 128], BF16)
            one_f32 = constp.tile([1, 128], F32)
            one_sb = constp.tile([1, 128], F32R)
            nc.vector.memset(one_f32[:], 1.0)
            nc.vector.tensor_copy(one_sb[:], one_f32[:])
            # Q^T/K^T per head, duplicated across both partition halves:
            # head h occupies free range [h*S, (h+1)*S) with the same [64, S]
            # data in partitions 0-63 and 64-127, so the scores matmuls can
            # run two tok_k chunks concurrently as row-tiles.
            qt2 = qkvp.tile([128, HPC * S], FP16)
            kt2 = qkvp.tile([128, HPC * S], FP16)
            # Natural V (bf16) with a ones column at col 64 of each 128-wide
            # per-head strip: the ctx matmul's M=65 stationary computes ctx
            # rows 0..63 plus the softmax denominator in row 64.
            v_sb = qkvp.tile([128, TC * HPC * 128], BF16)
            vt_sb = qkvp.tile([128, 2 * S], BF16)
            ctxf_sb = qkvp.tile([128, 2 * S], BF16)

            for t in range(TC):
                for h in range(HPC):
                    off = (t * HPC + h) * 128 + HD
                    nc.vector.memset(v_sb[:, off:off + 1], 1.0)


            # ---- phase 1: projections -------------------------------------
            with (
                tc.tile_pool(name="xw", bufs=1) as xwp,
                tc.tile_pool(name="ps1", bufs=2, space="PSUM") as ps1,
            ):
                xT_sb = xwp.tile([128, HC * S], FP16)
                wq_sb = xwp.tile([128, HC * QC], FP16)
                wk_sb = xwp.tile([128, HC * QC], FP16)
                wv_sb = xwp.tile([128, HC * QC], FP16)
                # wv + the first two xT chunks get the DMA engines to
                # themselves; later inputs are paced behind early V^T
                # matmuls (add_dep_helper) so the first compute isn't stuck
                # behind the whole 6 MB input load.
                xt_dmas = {}
                for hc in range(HC):
                    r = slice(hc * 128, (hc + 1) * 128)
                    nc.scalar.dma_start(wv_sb[:, hc * QC:(hc + 1) * QC],
                                        wv[r, :])
                    eng = nc.sync if hc % 2 == 0 else nc.scalar
                    if hc == 0:
                        # j-quartered so the first V^T matmul (which reads
                        # only tokens 0..511 of chunk 0) starts asap
                        for j in range(TB):
                            xt_dmas[hc] = eng.dma_start(
                                xT_sb[:, hc * S + j * 512:hc * S + (j + 1) * 512],
                                xT[r, j * 512:(j + 1) * 512])
                    else:
                        xt_dmas[hc] = eng.dma_start(
                            xT_sb[:, hc * S:(hc + 1) * S], xT[r, :])
                nc.scalar.dma_start(idf_sb[:, :], ident[:, :])
                nc.vector.tensor_copy(idb_sb[:, :], idf_sb[:, :])
                for ci in range(2):
                    nc.sync.dma_start(bq_sb[:, ci:ci + 1],
                                      bq[ci * 128:(ci + 1) * 128])
                    nc.sync.dma_start(bk_sb[:, ci:ci + 1],
                                      bk[ci * 128:(ci + 1) * 128])
                qk_dmas = []
                for hc in range(HC):
                    r = slice(hc * 128, (hc + 1) * 128)
                    qk_dmas.append(nc.sync.dma_start(
                        wq_sb[:, hc * QC:(hc + 1) * QC], wq[r, :]))
                    qk_dmas.append(nc.sync.dma_start(
                        wk_sb[:, hc * QC:(hc + 1) * QC], wk[r, :]))

                # V^T first (kept in SBUF; transposed on the PE below)
                vt_mms = {}
                for ci in range(2):
                    ps = ps1.tile([128, S], F32, tag="ps1")
                    for hc in range(HC):
                        for j in range(TB):
                            mm = nc.tensor.matmul(
                                ps[:, j * 512:(j + 1) * 512],
                                wv_sb[:, hc * QC + ci * 128:
                                      hc * QC + ci * 128 + 128],
                                xT_sb[:, hc * S + j * 512:
                                      hc * S + j * 512 + 512],
                                start=(hc == 0), stop=(hc == HC - 1))
                            vt_mms[(ci, hc, j)] = mm
                    nc.vector.tensor_copy(vt_sb[:, ci * S:(ci + 1) * S], ps[:])
                for hc in range(2, HC):
                    tile.add_dep_helper(xt_dmas[hc].ins, vt_mms[(0, hc - 2, 3)].ins,
                                        reason="pace xT input load")
                for i, d in enumerate(qk_dmas):
                    src_mm = vt_mms[(0, min(i // 2, HC - 1), 1)]
                    tile.add_dep_helper(d.ins, src_mm.ins, reason="pace w input load")
                for ci in range(2):
                    d = nc.scalar.dma_start(
                        wo_sb[:, ci * HID:(ci + 1) * HID],
                        wo[ci * 128:(ci + 1) * 128, :])
                    tile.add_dep_helper(d.ins, vt_mms[(1, 3 + 2 * ci, 0)].ins,
                                        reason="pace wo load")

                # Q^T and K^T, written into the duplicated per-head layout
                for ci in range(2):
                    for w_sb, b_sb, dst in ((wq_sb, bq_sb, qt2),
                                            (wk_sb, bk_sb, kt2)):
                        ps = ps1.tile([128, S], F32, tag="ps1")
                        for hc in range(HC):
                            for j in range(TB):
                                nc.tensor.matmul(
                                    ps[:, j * 512:(j + 1) * 512],
                                    w_sb[:, hc * QC + ci * 128:
                                         hc * QC + ci * 128 + 128],
                                    xT_sb[:, hc * S + j * 512:
                                          hc * S + j * 512 + 512],
                                    start=(hc == 0), stop=(hc == HC - 1))
                        hA, hB = 2 * ci, 2 * ci + 1
                        nc.vector.tensor_scalar_add(
                            dst[0:64, hA * S:(hA + 1) * S], ps[0:64, :],
                            b_sb[0:64, ci:ci + 1])
                        nc.vector.tensor_scalar_add(
                            dst[64:128, hB * S:(hB + 1) * S], ps[64:128, :],
                            b_sb[64:128, ci:ci + 1])
                        nc.sync.dma_start(dst[64:128, hA * S:(hA + 1) * S],
                                          dst[0:64, hA * S:(hA + 1) * S])
                        nc.scalar.dma_start(dst[0:64, hB * S:(hB + 1) * S],
                                            dst[64:128, hB * S:(hB + 1) * S])

            # V^T -> V via PE transpose-mode ([128,128] pair tiles), then a
            # strided DVE copy into the ones-padded layout.
            with tc.tile_pool(name="trp", bufs=4, space="PSUM") as trp:
                for ci in range(2):
                    for t in range(TC):
                        tp = trp.tile([128, 128], BF16, tag="tr")
                        nc.tensor.transpose(
                            tp[:, :],
                            vt_sb[:, ci * S + t * 128:ci * S + t * 128 + 128],
                            idb_sb[:, :])
                        dst = v_sb[:, (t * HPC + 2 * ci) * 128:
                                   (t * HPC + 2 * ci + 2) * 128].rearrange(
                            "p (h e) -> p h e", h=2)[:, :, 0:HD]
                        srcv = tp[:, :].rearrange("p (h e) -> p h e", h=2)
                        nc.vector.tensor_copy(dst, srcv)

            # ---- phase 2: attention per head ------------------------------
            with (
                tc.tile_pool(name="probs", bufs=5) as probsp,
                tc.tile_pool(name="craw", bufs=2) as crawp,
                tc.tile_pool(name="div", bufs=2) as divp,
                tc.tile_pool(name="scps", bufs=2, space="PSUM") as scps,
                tc.tile_pool(name="ctps", bufs=1, space="PSUM") as ctps,
            ):
                NCP = TC // 2
                heads = (1, 3, 0, 2)
                stages = [(h, cp) for h in heads for cp in range(NCP)]
                probs_tiles = {}
                ctx_tiles = {}

                def emit_scores(h, cp):
                    hS = h * S
                    c0, c1 = 2 * cp, 2 * cp + 1
                    probs_c = probsp.tile([128, 2 * S], BF16, tag="probs",
                                          name=f"probs_h{h}_cp{cp}")
                    probs_tiles[(h, cp)] = probs_c
                    for j in range(TB):
                        sp = scps.tile([128, 1024], F32, tag="sc")
                        nc.tensor.matmul(
                            sp[:, 0:512],
                            kt2[0:64, hS + c0 * 128:hS + c0 * 128 + 128],
                            qt2[0:64, hS + j * 512:hS + j * 512 + 512],
                            start=True, stop=True)
                        nc.tensor.matmul(
                            sp[:, 512:1024],
                            kt2[64:128, hS + c1 * 128:hS + c1 * 128 + 128],
                            qt2[64:128, hS + j * 512:hS + j * 512 + 512],
                            start=True, stop=True)
                        nc.scalar.activation(
                            probs_c[:, j * 1024:(j + 1) * 1024], sp[:, :], EXP)

                def emit_ctx(h, cp):
                    c0, c1 = 2 * cp, 2 * cp + 1
                    if cp == 0:
                        ctx_tiles[h] = ctps.tile([128, S], F32, tag="ctx", name=f"ctx_ps_h{h}")
                    ctx_ps = ctx_tiles[h]
                    probs_c = probs_tiles.pop((h, cp))
                    for j in range(TB):
                        for ck, coff in ((c0, 0), (c1, 512)):
                            vbase = (ck * HPC + h) * 128
                            nc.tensor.matmul(
                                ctx_ps[0:65, j * 512:(j + 1) * 512],
                                v_sb[:, vbase:vbase + 65],
                                probs_c[:, j * 1024 + coff:
                                        j * 1024 + coff + 512],
                                start=(cp == 0 and ck == c0),
                                stop=(cp == NCP - 1 and ck == c1))

                def emit_division(h):
                    ci = h // 2
                    ctx_ps = ctx_tiles.pop(h)
                    last = (h == 2)
                    if last:
                        craw = crawp.tile([128, S], F32, tag="craw")
                        nc.vector.tensor_copy(craw[0:64, :], ctx_ps[0:64, :])
                        drow = divp.tile([1, S], F32, tag="drow")
                        nc.scalar.copy(drow[0:1, :], ctx_ps[64:65, :])
                        denr = divp.tile([128, 16], F32, tag="denr")
                        nc.gpsimd.dma_start(denr[:, :], drow[0:1, :])
                        recr = divp.tile([128, 16], F32, tag="recr")
                        nc.vector.reciprocal(recr[:], denr[:])
                        rrow = divp.tile([1, S], F32, tag="rrow")
                        nc.gpsimd.dma_start(rrow[:, :], recr[:, :])
                        Dt = divp.tile([128, S], F32, tag="Dt")
                        for j in range(TB):
                            jj = slice(j * 512, (j + 1) * 512)
                            nc.gpsimd.partition_broadcast(Dt[:, jj],
                                                          rrow[0:1, jj])
                            nc.vector.tensor_tensor(
                                out=ctxf_sb[0:64, ci * S + j * 512:
                                            ci * S + (j + 1) * 512],
                                in0=craw[0:64, jj], in1=Dt[0:64, jj], op=MULT)
                    else:
                        craw = crawp.tile([128, S], F32, tag="craw")
                        nc.vector.tensor_copy(craw[0:65, :], ctx_ps[0:65, :])
                        denr = divp.tile([128, 16], F32, tag="denr")
                        nc.sync.dma_start(denr[:, :], craw[64:65, :])
                        recr = divp.tile([128, 16], F32, tag="recr")
                        nc.vector.reciprocal(recr[:], denr[:])
                        rrow = divp.tile([1, S], F32, tag="rrow")
                        nc.sync.dma_start(rrow[:, :], recr[:, :])
                        Dt = divp.tile([128, S], F32, tag="Dt")
                        nc.gpsimd.partition_broadcast(Dt[:, :], rrow[0:1, :])
                        if h % 2 == 0:
                            nc.vector.tensor_tensor(
                                out=ctxf_sb[0:64, ci * S:(ci + 1) * S],
                                in0=craw[0:64, :], in1=Dt[0:64, :], op=MULT)
                        else:
                            ctxd = crawp.tile([64, S], BF16, tag="ctxd")
                            nc.vector.tensor_tensor(
                                out=ctxd[0:64, :],
                                in0=craw[0:64, :], in1=Dt[0:64, :], op=MULT)
                            nc.sync.dma_start(
                                ctxf_sb[64:128, ci * S:(ci + 1) * S],
                                ctxd[0:64, :])

                # software pipeline: scores/exp lead ctx by LEAD cp-steps so
                # the PE stream keeps flowing across head boundaries
                LEAD = 2
                for i in range(len(stages) + LEAD):
                    if i < len(stages):
                        emit_scores(*stages[i])
                    if i >= LEAD:
                        h, cp = stages[i - LEAD]
                        emit_ctx(h, cp)
                        if cp == NCP - 1:
                            emit_division(h)

            # ---- phase 3: out projection (partial sums) -------------------
            with (
                tc.tile_pool(name="ops", bufs=8, space="PSUM") as ops,
                tc.tile_pool(name="ostg", bufs=3) as ostg,
            ):
                for t in range(TC):
                    op0 = ops.tile([128, 512], F32, tag="op")
                    op1 = ops.tile([128, 512], F32, tag="op")
                    for ci in range(2):
                        for oc, op in ((0, op0), (1, op1)):
                            nc.tensor.matmul(
                                op[:, :],
                                ctxf_sb[:, ci * S + t * 128:ci * S + t * 128 + 128],
                                wo_sb[:, ci * HID + oc * 512:
                                      ci * HID + oc * 512 + 512],
                                start=(ci == 0), stop=(ci == 1))
                    ot = ostg.tile([128, 1024], BF16, tag="ot")
                    nc.scalar.copy(ot[:, 0:512], op0[:, :])
                    nc.vector.tensor_copy(ot[:, 512:1024], op1[:, :])
                    nc.sync.dma_start(out[t * 128:(t + 1) * 128, :], ot[:, :])

    nc.compile()
    return nc


_NC = None


def _get_nc():
    global _NC
    if _NC is None:
        _NC = build_nc()
    return _NC


def make_in_maps(x, Wq, bq, Wk, bk, Wv, bv, Wo, bo):
    in_maps = []
    for core in range(NCORES):
        b, g = core // 4, core % 4
        sl = slice(g * QC, (g + 1) * QC)
        in_maps.append({
            "xT": np.ascontiguousarray(x[b].T).astype(np.float16),
            "wq": (np.ascontiguousarray(Wq[:, sl]) * 0.125).astype(np.float16),
            "wk": np.ascontiguousarray(Wk[:, sl]).astype(np.float16),
            "wv": np.ascontiguousarray(Wv[:, sl]).astype(np.float16),
            "wo": np.ascontiguousarray(Wo[sl, :]).astype(ml_dtypes.bfloat16),
            "bq": (np.asarray(bq[sl]) * 0.125).astype(np.float32),
            "bk": np.asarray(bk[sl]).astype(np.float32),
            "ident": np.eye(128, dtype=np.float32),
        })
    return in_maps


def combine_outputs(core_outs, Wv_bias_term):
    full = np.empty((B, S, HID), np.float32)
    for b in range(B):
        acc = core_outs[4 * b].astype(np.float32).copy()
        for g in range(1, 4):
            acc += core_outs[4 * b + g]
        full[b] = acc + Wv_bias_term
    return full


def kernel(**inputs):
    x = np.asarray(inputs["x"], np.float32)
    Wq = np.asarray(inputs["Wq"], np.float32)
    bq = np.asarray(inputs["bq"], np.float32)
    Wk = np.asarray(inputs["Wk"], np.float32)
    bk = np.asarray(inputs["bk"], np.float32)
    Wv = np.asarray(inputs["Wv"], np.float32)
    bv = np.asarray(inputs["bv"], np.float32)
    Wo = np.asarray(inputs["Wo"], np.float32)
    bo = np.asarray(inputs["bo"], np.float32)

    nc = _get_nc()
    in_maps = make_in_maps(x, Wq, bq, Wk, bk, Wv, bv, Wo, bo)
    res = run_bass_kernel_spmd(nc, in_maps, core_ids=list(range(NCORES)))
    core_outs = [res.results[c]["out"] for c in range(NCORES)]
    bias_term = (bv @ Wo + bo).astype(np.float32)
    return combine_outputs(core_outs, bias_term)
